# revision 1
# baseline (speedup 1.0000x reference)
"""AttnBlock (GroupNorm + single-head spatial self-attention + residual) on 8 TRN2 cores.

Sharding: data-parallel over batch — B=16 images, 2 per NeuronCore. Each core runs
an identical Bass/Tile program over its 2 images; no cross-core communication.

Per-image pipeline (all on one core, C=512 channels, HW=1024 spatial):
  1. GroupNorm(32 groups): per-channel sum/sumsq (DVE/ACT), group-combine via a
     tiny matmul with a 0/1 group-selector, broadcast back via its transpose.
  2. q,k (C x HW, channel-partitioned) and vT (HW x C, spatial-partitioned)
     via 1x1-conv matmuls against pre-transposed weights.
  3. scores^T[j,i] = sum_c k[c,j] q[c,i]; exp (with the C^-0.5 scale folded into
     the ACT activation) -> P^T; den[i] = sum_j P^T via ones-matmul.
  4. num[c,i] = sum_j vT[j,c] P^T[j,i]; proj = woT.T @ num.
  5. out = x + bo + proj * (1/den)  (softmax normalization commutes with the
     channel-wise output projection, so it is applied once at the end).

The attention internals run in bf16 (matmul operands; fp32 PSUM accumulation).
The residual path (x, GroupNorm stats, final add) stays fp32; measured end-to-end
error vs the fp32 reference is ~3e-5 relative.
"""

import numpy as np
import ml_dtypes
from contextlib import ExitStack

import concourse.bass as bass
import concourse.bacc as bacc
import concourse.tile as tile
import concourse.mybir as mybir
from concourse.bass_utils import run_bass_kernel_spmd

F32 = mybir.dt.float32
AF = mybir.ActivationFunctionType
OP = mybir.AluOpType
AX = mybir.AxisListType

B, C, H, W = 16, 512, 32, 32
HW = H * W            # 1024
G = 32                # groupnorm groups
CPG = C // G          # 16 channels per group
EPS = 1e-5
NCORES = 8
BPC = B // NCORES     # 2 images per core
P = 128               # SBUF partitions
NCT = C // P          # 4 channel tiles
GPT = P // CPG        # 8 groups per channel tile
NSB = HW // P         # 8 spatial blocks of 128
FC = 512              # matmul moving-dim chunk (one PSUM bank of fp32)
NIC = HW // FC        # 2 chunks over the spatial free dim
SM_SCALE = float(C) ** -0.5

# Attention-internals dtype. bf16 keeps SBUF small and matmuls at 1 cycle/row.
DT = mybir.dt.bfloat16
DT_NP = ml_dtypes.bfloat16

_CACHE: dict = {}


def _mm(nc, out, lhsT, rhs, start, stop):
    nc.tensor.matmul(out, lhsT, rhs, start=start, stop=stop)


def _emit(ctx, tc, io):
    nc = tc.nc

    consts = ctx.enter_context(tc.tile_pool(name="consts", bufs=1))
    pX16 = ctx.enter_context(tc.tile_pool(name="pX16", bufs=2))
    pX = ctx.enter_context(tc.tile_pool(name="pX", bufs=2))
    pHN = ctx.enter_context(tc.tile_pool(name="pHN", bufs=2))
    pQ = ctx.enter_context(tc.tile_pool(name="pQ", bufs=1))
    pK = ctx.enter_context(tc.tile_pool(name="pK", bufs=1))
    pVT = ctx.enter_context(tc.tile_pool(name="pVT", bufs=1))
    pPT = ctx.enter_context(tc.tile_pool(name="pPT", bufs=1))
    pNUM = ctx.enter_context(tc.tile_pool(name="pNUM", bufs=1))
    pOUT = ctx.enter_context(tc.tile_pool(name="pOUT", bufs=2))
    pS = ctx.enter_context(tc.tile_pool(name="pS", bufs=2))
    pmm = ctx.enter_context(tc.tile_pool(name="pmm", bufs=4, space="PSUM"))
    paux = ctx.enter_context(tc.tile_pool(name="paux", bufs=2, space="PSUM"))
    ptiny = ctx.enter_context(tc.tile_pool(name="ptiny", bufs=2, space="PSUM"))

    # ---- image 0's x (bf16 copy) first: it gates the whole pipeline. Only
    # GroupNorm stats + hn read it, so bf16 halves the gating bytes; the fp32
    # x needed for the residual add arrives much later. Split across both
    # HWDGE queues (sync + scalar); everything else queues behind it on sync.
    X16_0 = pX16.tile([P, NCT, HW], DT, name="X16_0", tag="X16")
    for ct in range(NCT):
        (nc.sync if ct % 2 == 0 else nc.scalar).dma_start(
            X16_0[:, ct, :], io["x16"][0, ct * P:(ct + 1) * P, :])

    def load_const(name, shape, dtype=F32):
        t = consts.tile(list(shape), dtype, name=f"c_{name}")
        nc.sync.dma_start(t[:], io[name][:])
        return t

    # all (P, *) vectors packed into ONE DMA — each dma_start costs ~600ns of
    # sync-engine descriptor time that would otherwise delay the weight loads
    cvec = load_const("cvec", (P, 5 * NCT + GPT))
    bq_sb = cvec[:, 0 * NCT:1 * NCT]
    bk_sb = cvec[:, 1 * NCT:2 * NCT]
    bo_sb = cvec[:, 2 * NCT:3 * NCT]
    gs_sb = cvec[:, 3 * NCT:4 * NCT]
    gb_sb = cvec[:, 4 * NCT:5 * NCT]
    gsel = cvec[:, 5 * NCT:5 * NCT + GPT]
    gselT = load_const("gselT", (GPT, P))
    bv_r = load_const("bv_r", (1, C))

    # ---- weights (loaded once, shared by both images), one packed DMA per
    # matrix: 4 descriptors instead of 16 (~600ns of sync-engine time each)
    w_sb = {}
    for wname in ("wqt", "wkt", "wvt", "wot"):
        t = consts.tile([P, NCT * C], DT, name=f"{wname}_p")
        nc.sync.dma_start(t[:], io[wname][:])
        w_sb[wname] = t

    def wsl(wname, ct, lo, hi):
        # column block [lo:hi) of the ct-th c_in tile of a packed weight
        return w_sb[wname][:, ct * C + lo:ct * C + hi]

    ones_col = consts.tile([P, 1], DT, name="ones_col")
    nc.vector.memset(ones_col[:], 1.0)
    ones_row = consts.tile([1, P], DT, name="ones_row")
    nc.vector.memset(ones_row[:], 1.0)
    zb = consts.tile([P, 1], F32, name="zb")
    nc.vector.memset(zb[:], 0.0)
    epsb = consts.tile([GPT, 1], F32, name="epsb")
    nc.vector.memset(epsb[:], EPS)

    # bv broadcast to all partitions: ones_row.T @ bv_r  (K=1 matmul)
    bv_rdt = consts.tile([1, C], DT, name="bv_rdt")
    nc.vector.tensor_copy(bv_rdt[:], bv_r[:])
    bvb_ps = pmm.tile([P, C], F32, name="bvb_ps", tag="mm")
    _mm(nc, bvb_ps[:], ones_row[:], bv_rdt[:], start=True, stop=True)
    bv_b = consts.tile([P, C], F32, name="bv_b")
    nc.vector.tensor_copy(bv_b[:], bvb_ps[:])

    # ---- per-image emission ----
    def new_img(i):
        return {"i": i}

    def emit_load16(im):
        i = im["i"]
        if i == 0:
            im["X16"] = X16_0
            return
        X16 = pX16.tile([P, NCT, HW], DT, name=f"X16_{i}", tag="X16")
        for ct in range(NCT):
            nc.sync.dma_start(X16[:, ct, :], io["x16"][i, ct * P:(ct + 1) * P, :])
        im["X16"] = X16

    def emit_load32(im):
        # host-packed to (P, NCT*HW): one descriptor per image; only the final
        # residual adds read it, so the coarser completion semaphore is free
        i = im["i"]
        X = pX.tile([P, NCT, HW], F32, name=f"X{i}", tag="X")
        nc.sync.dma_start(X[:, :, :], io["x"][i])
        im["X"] = X

    def emit_stats(im):
        i = im["i"]
        X16 = im["X16"]
        stats = pS.tile([P, 2 * NCT], F32, name=f"stats{i}", tag="stats")
        scratch = pS.tile([P, HW], DT, name=f"scr{i}", tag="scratch")
        for ct in range(NCT):
            nc.vector.tensor_reduce(stats[:, ct:ct + 1], X16[:, ct, :], AX.X, OP.add)
            nc.scalar.activation(scratch[:], X16[:, ct, :], AF.Square, bias=zb[:],
                                 accum_out=stats[:, NCT + ct:NCT + ct + 1])
        im["stats"] = stats

    def emit_norm(im):
        i = im["i"]
        X16, stats = im["X16"], im["stats"]
        with nc.named_scope(f"norm{i}"):
            gst = ptiny.tile([GPT, 2 * NCT], F32, name=f"gst{i}", tag="tiny")
            _mm(nc, gst[:], gsel[:], stats[:], start=True, stop=True)
            gm = pS.tile([GPT, 2 * NCT], F32, name=f"gm{i}", tag="gm")
            nc.vector.tensor_scalar_mul(gm[:], gst[:], 1.0 / (CPG * HW))
            sq = pS.tile([GPT, NCT], F32, name=f"sq{i}", tag="sq")
            nc.vector.tensor_mul(sq[:], gm[:, 0:NCT], gm[:, 0:NCT])
            var = pS.tile([GPT, NCT], F32, name=f"var{i}", tag="var")
            nc.vector.tensor_sub(var[:], gm[:, NCT:], sq[:])
            std = pS.tile([GPT, NCT], F32, name=f"std{i}", tag="std")
            nc.scalar.activation(std[:], var[:], AF.Sqrt, bias=epsb[:])
            gmr = pS.tile([GPT, 2 * NCT], F32, name=f"gmr{i}", tag="gmr")
            nc.vector.tensor_copy(gmr[:, 0:NCT], gm[:, 0:NCT])
            nc.vector.reciprocal(gmr[:, NCT:], std[:])
            pmr = ptiny.tile([P, 2 * NCT], F32, name=f"pmr{i}", tag="tiny")
            _mm(nc, pmr[:], gselT[:], gmr[:], start=True, stop=True)
            mr = pS.tile([P, 2 * NCT], F32, name=f"mr{i}", tag="mr")
            nc.vector.tensor_copy(mr[:], pmr[:])
            # a = rstd*scale (cols NCT..), b = gn_bias - mean*a (cols 0..NCT)
            ab = pS.tile([P, 2 * NCT], F32, name=f"ab{i}", tag="ab")
            tb = pS.tile([P, NCT], F32, name=f"tb{i}", tag="tb")
            for ct in range(NCT):
                a_col = ab[:, NCT + ct:NCT + ct + 1]
                nc.vector.tensor_mul(a_col, mr[:, NCT + ct:NCT + ct + 1], gs_sb[:, ct:ct + 1])
                nc.vector.tensor_mul(tb[:, ct:ct + 1], mr[:, ct:ct + 1], a_col)
                nc.vector.tensor_sub(ab[:, ct:ct + 1], gb_sb[:, ct:ct + 1], tb[:, ct:ct + 1])
            HN = pHN.tile([P, NCT, HW], DT, name=f"HN{i}", tag="HN")
            for ct in range(NCT):
                nc.vector.tensor_scalar(HN[:, ct, :], X16[:, ct, :],
                                        ab[:, NCT + ct:NCT + ct + 1], ab[:, ct:ct + 1],
                                        OP.mult, OP.add)
            im["HN"] = HN

    def emit_qkv(im):
        i = im["i"]
        HN = im["HN"]
        with nc.named_scope(f"qkv{i}"):
            Q = pQ.tile([P, NCT, HW], DT, name=f"Q{i}", tag="Q")
            K = pK.tile([P, NCT, HW], DT, name=f"K{i}", tag="K")
            for wname, bias_sb, OT in (("wqt", bq_sb, Q), ("wkt", bk_sb, K)):
                for ob in range(NCT):
                    ps = [pmm.tile([P, FC], F32, name=f"{wname}ps{i}_{ob}_{ic}", tag="mm")
                          for ic in range(NIC)]
                    for ct in range(NCT):
                        lhs = wsl(wname, ct, ob * P, (ob + 1) * P)
                        for ic in range(NIC):
                            _mm(nc, ps[ic][:], lhs, HN[:, ct, ic * FC:(ic + 1) * FC],
                                start=(ct == 0), stop=(ct == NCT - 1))
                    for ic in range(NIC):
                        nc.scalar.add(OT[:, ob, ic * FC:(ic + 1) * FC], ps[ic][:],
                                      bias_sb[:, ob:ob + 1])
            VT = pVT.tile([P, NSB, C], DT, name=f"VT{i}", tag="VT")
            for sb in range(NSB):
                ps = pmm.tile([P, C], F32, name=f"vtps{i}_{sb}", tag="mm")
                for ct in range(NCT):
                    _mm(nc, ps[:], HN[:, ct, sb * P:(sb + 1) * P], wsl("wvt", ct, 0, C),
                        start=(ct == 0), stop=(ct == NCT - 1))
                nc.vector.tensor_add(VT[:, sb, :], ps[:], bv_b[:])
            im["Q"], im["K"], im["VT"] = Q, K, VT

    def emit_scores(im):
        i = im["i"]
        Q, K = im["Q"], im["K"]
        with nc.named_scope(f"scores{i}"):
            PT = pPT.tile([P, NSB, HW], DT, name=f"PT{i}", tag="PT")
            for jb in range(NSB):
                ps = [pmm.tile([P, FC], F32, name=f"sps{i}_{jb}_{ic}", tag="mm")
                      for ic in range(NIC)]
                for ct in range(NCT):
                    lhs = K[:, ct, jb * P:(jb + 1) * P]
                    for ic in range(NIC):
                        _mm(nc, ps[ic][:], lhs, Q[:, ct, ic * FC:(ic + 1) * FC],
                            start=(ct == 0), stop=(ct == NCT - 1))
                for ic in range(NIC):
                    nc.scalar.activation(PT[:, jb, ic * FC:(ic + 1) * FC], ps[ic][:],
                                         AF.Exp, bias=zb[:], scale=SM_SCALE)
            recip = pS.tile([1, HW], F32, name=f"recip{i}", tag="recip")
            recip_dt = pS.tile([1, HW], DT, name=f"recipdt{i}", tag="recipdt")
            for ic in range(NIC):
                den = paux.tile([1, FC], F32, name=f"den{i}_{ic}", tag="aux")
                for jb in range(NSB):
                    _mm(nc, den[:], ones_col[:], PT[:, jb, ic * FC:(ic + 1) * FC],
                        start=(jb == 0), stop=(jb == NSB - 1))
                sl = slice(ic * FC, (ic + 1) * FC)
                nc.vector.reciprocal(recip[:, sl], den[:])
                nc.vector.tensor_copy(recip_dt[:, sl], recip[:, sl])
            im["PT"], im["recip"] = PT, recip_dt

    def emit_attn_out(im):
        i = im["i"]
        X, VT, PT = im["X"], im["VT"], im["PT"]
        with nc.named_scope(f"attnout{i}"):
            # num = vT.T @ P^T with the 1/den softmax normalization folded into
            # the PSUM eviction (commutes with the channel-wise wo projection)
            recipb = pS.tile([P, HW], F32, name=f"recipb{i}", tag="recipb")

            def emit_rb(ic):
                rb = paux.tile([P, FC], F32, name=f"rb{i}_{ic}", tag="aux")
                _mm(nc, rb[:], ones_row[:], im["recip"][:, ic * FC:(ic + 1) * FC],
                    start=True, stop=True)
                nc.vector.tensor_copy(recipb[:, ic * FC:(ic + 1) * FC], rb[:])

            emit_rb(0)
            NUM = pNUM.tile([P, NCT, HW], DT, name=f"NUM{i}", tag="NUM")
            for cb in range(NCT):
                ps = [pmm.tile([P, FC], F32, name=f"nps{i}_{cb}_{ic}", tag="mm")
                      for ic in range(NIC)]
                for jt in range(NSB):
                    lhs = VT[:, jt, cb * P:(cb + 1) * P]
                    for ic in range(NIC):
                        _mm(nc, ps[ic][:], lhs, PT[:, jt, ic * FC:(ic + 1) * FC],
                            start=(jt == 0), stop=(jt == NSB - 1))
                if cb == 0:
                    emit_rb(1)  # cb0's matmuls cover the ic1 recip chain latency
                for ic in range(NIC):
                    sl = slice(ic * FC, (ic + 1) * FC)
                    nc.vector.tensor_mul(NUM[:, cb, sl], ps[ic][:], recipb[:, sl])
            # proj + residual (+bo) straight from PSUM, then store
            OUTT = pOUT.tile([P, NCT, HW], F32, name=f"OUT{i}", tag="OUT")
            for ob in range(NCT):
                ps = [pmm.tile([P, FC], F32, name=f"pps{i}_{ob}_{ic}", tag="mm")
                      for ic in range(NIC)]
                for ct in range(NCT):
                    lhs = wsl("wot", ct, ob * P, (ob + 1) * P)
                    for ic in range(NIC):
                        _mm(nc, ps[ic][:], lhs, NUM[:, ct, ic * FC:(ic + 1) * FC],
                            start=(ct == 0), stop=(ct == NCT - 1))
                for ic in range(NIC):
                    sl = slice(ic * FC, (ic + 1) * FC)
                    nc.vector.scalar_tensor_tensor(OUTT[:, ob, sl], ps[ic][:],
                                                   bo_sb[:, ob:ob + 1], X[:, ob, sl],
                                                   OP.add, OP.add)
                    (nc.sync if ic == 0 else nc.scalar).dma_start(
                        io["out"][i, ob * P:(ob + 1) * P, sl], OUTT[:, ob, sl])

    ims = [new_img(i) for i in range(BPC)]
    a, b = ims
    emit_load16(a)
    emit_stats(a)
    emit_load16(b)
    emit_stats(b)
    emit_norm(a)
    emit_load32(a)
    emit_qkv(a)
    emit_norm(b)
    emit_load32(b)
    emit_scores(a)
    emit_attn_out(a)
    emit_qkv(b)
    emit_scores(b)
    emit_attn_out(b)


def _build():
    if "nc" in _CACHE:
        return _CACHE["nc"]
    nc = bacc.Bacc("TRN2", target_bir_lowering=False, debug=False, num_devices=NCORES)
    io = {}
    io["x"] = nc.dram_tensor("x", [BPC, P, NCT * HW], F32, kind="ExternalInput").ap()
    io["x16"] = nc.dram_tensor("x16", [BPC, C, HW], DT, kind="ExternalInput").ap()
    for wname in ("wqt", "wkt", "wvt", "wot"):
        io[wname] = nc.dram_tensor(wname, [P, NCT * C], DT, kind="ExternalInput").ap()
    io["cvec"] = nc.dram_tensor("cvec", [P, 5 * NCT + GPT], F32,
                                kind="ExternalInput").ap()
    io["bv_r"] = nc.dram_tensor("bv_r", [1, C], F32, kind="ExternalInput").ap()
    io["gselT"] = nc.dram_tensor("gselT", [GPT, P], F32, kind="ExternalInput").ap()
    io["out"] = nc.dram_tensor("out", [BPC, C, HW], F32, kind="ExternalOutput").ap()

    with tile.TileContext(nc) as tc:
        with ExitStack() as ctx:
            _emit(ctx, tc, io)
    nc.compile()
    _CACHE["nc"] = nc
    return nc


def _col_layout(v):
    # (C,) -> (P, NCT): column ct holds channels [ct*128, (ct+1)*128)
    return np.ascontiguousarray(np.asarray(v, np.float32).reshape(NCT, P).T)


def _run(inputs, trace=False, **run_kwargs):
    x = np.ascontiguousarray(np.asarray(inputs["x"], np.float32).reshape(B, C, HW))
    def _wpack(w):
        # wT (c_in, c_out) -> (P, NCT*C): W[p, ct*C + j] = wT[ct*128+p, j]
        wt = np.asarray(w, np.float32).T.astype(DT_NP)
        return np.ascontiguousarray(
            wt.reshape(NCT, P, C).transpose(1, 0, 2).reshape(P, NCT * C))

    wdt = {n: _wpack(inputs[s])
           for n, s in (("wqt", "wq"), ("wkt", "wk"), ("wvt", "wv"), ("wot", "wo"))}
    pidx = np.arange(P)
    gsel = (pidx[:, None] // CPG == np.arange(GPT)[None, :]).astype(np.float32)
    cvec = np.concatenate([_col_layout(inputs["bq"]), _col_layout(inputs["bk"]),
                           _col_layout(inputs["bo"]), _col_layout(inputs["gn_scale"]),
                           _col_layout(inputs["gn_bias"]), gsel], axis=1)
    common = {
        **wdt,
        "cvec": np.ascontiguousarray(cvec),
        "bv_r": np.ascontiguousarray(np.asarray(inputs["bv"], np.float32).reshape(1, C)),
        "gselT": np.ascontiguousarray(gsel.T),
    }
    x16 = x.astype(DT_NP)
    # x packed to (BPC, P, NCT*HW) to match the single-descriptor load
    xp = x.reshape(B, NCT, P, HW).transpose(0, 2, 1, 3).reshape(B, P, NCT * HW)
    in_maps = [{"x": np.ascontiguousarray(xp[m * BPC:(m + 1) * BPC]),
                "x16": np.ascontiguousarray(x16[m * BPC:(m + 1) * BPC]), **common}
               for m in range(NCORES)]
    nc = _build()
    res = run_bass_kernel_spmd(nc, in_maps, core_ids=list(range(NCORES)),
                               trace=trace, **run_kwargs)
    out = np.concatenate([r["out"] for r in res.results], axis=0)
    return out.reshape(B, C, H, W).astype(np.float32), res


def kernel(**inputs):
    out, _ = _run(inputs)
    return out



# revision 9
# speedup vs baseline: 1.5548x; 1.5548x over previous
"""AttnBlock (GroupNorm + single-head spatial self-attention + residual) on 8 TRN2 cores.

Sharding: data-parallel over batch — B=16 images, 2 per NeuronCore. Each core runs
an identical Bass/Tile program over its 2 images; no cross-core communication.

All matmuls run in fp8 (e4m3) with DoubleRow perf mode: each matmul contracts
K=256 (two 128-partition k-tiles) at 2 rows/cycle — 2x the bf16 tensor-engine
throughput. fp32 PSUM accumulation throughout.

Scale management (e4m3 normals start at 2^-6, so small tensors must be
pre-scaled; all scales are powers of two and cancel exactly):
  - weights wq,wk,wv,wo are stored as fp8(32*w)         (w sigma ~0.02)
  - hn (GroupNorm output, sigma 1) is unscaled fp8
  - q,k carry x32; the exp() activation scale folds 1/(32*32) with C^-0.5
  - v carries x32; NUM = num_psum * (2/den) carries 64x (num sigma ~0.013)
  - proj psum carries 32*64 = 2048x; the residual x is pre-scaled by 2048
    host-side (exact in bf16), so the final eviction is a single
    scalar_tensor_tensor and the host divides the bf16 output by 2048.

GroupNorm runs on x~ = 2048*x in bf16: passing eps~ = 2048^2*eps makes the
whole stats pipeline scale-invariant; the per-channel affine a,b come out
already in x~ units. Residual and output stay bf16 (the attention branch is
~0.6% of the output norm, so end-to-end error is ~2.4e-3, dominated by the
bf16 residual/output quantization, with an 8x margin to the 2e-2 gate).

The v bias never appears on-chip: since sum_j P[j,i] = den[i] holds exactly for
the stored PT8, bv contributes wo @ bv to the output, which is folded into bo
host-side. The V eviction is then a pure PSUM->SBUF copy.

Engine split per image (PE is the bottleneck at ~29us/image; GPSIMD cannot
access PSUM, so all PSUM evictions live on ACT/DVE):
  ACT  : x^2 stats (accum), exp->PT8, q eviction (+bias), V eviction (copy)
  DVE  : sum stats, k eviction, NUM eviction (*recip), OUT eviction (+resid),
         reciprocal chain, GroupNorm combine glue, 1 hn tile
  Pool : hn tiles (SBUF->SBUF)
"""

import numpy as np
import ml_dtypes
from contextlib import ExitStack

import concourse.bass as bass
import concourse.bacc as bacc
import concourse.tile as tile
import concourse.mybir as mybir
from concourse.bass_utils import run_bass_kernel_spmd

F32 = mybir.dt.float32
AF = mybir.ActivationFunctionType
OP = mybir.AluOpType
AX = mybir.AxisListType
PM = mybir.MatmulPerfMode

B, C, H, W = 16, 512, 32, 32
HW = H * W            # 1024
G = 32                # groupnorm groups
CPG = C // G          # 16 channels per group
EPS = 1e-5
NCORES = 8
BPC = B // NCORES     # 2 images per core
P = 128               # SBUF partitions
NCT = C // P          # 4 channel tiles
NKP = NCT // 2        # 2 k-tile pairs for DoubleRow over C
GPT = P // CPG        # 8 groups per channel tile
NSB = HW // P         # 8 spatial blocks of 128
NJP = NSB // 2        # 4 k-tile pairs for DoubleRow over HW
FC = 512              # matmul moving-dim chunk (one PSUM bank of fp32)
NIC = HW // FC        # 2 chunks over the spatial free dim
SM_SCALE = float(C) ** -0.5

SW = 32.0             # weight scale (all four conv weights)
SR = 2.0              # extra scale folded into recip (NUM carries SW*SR=64)
SX = SW * SW * SR     # 2048: residual/output scale
EPS_T = SX * SX * EPS # eps for stats computed on 2048*x

DT = mybir.dt.bfloat16
DT_NP = ml_dtypes.bfloat16
F8 = mybir.dt.float8e4
F8_NP = ml_dtypes.float8_e4m3

_CACHE: dict = {}


def _mm8(nc, out, lhsT, rhs, start, stop):
    nc.tensor.matmul(out, lhsT, rhs, start=start, stop=stop,
                     perf_mode=PM.DoubleRow)


def _emit(ctx, tc, io):
    nc = tc.nc

    consts = ctx.enter_context(tc.tile_pool(name="consts", bufs=1))
    pX16 = ctx.enter_context(tc.tile_pool(name="pX16", bufs=2))
    pHN = ctx.enter_context(tc.tile_pool(name="pHN", bufs=2))
    pQ = ctx.enter_context(tc.tile_pool(name="pQ", bufs=1))
    pK = ctx.enter_context(tc.tile_pool(name="pK", bufs=1))
    pVT = ctx.enter_context(tc.tile_pool(name="pVT", bufs=2))
    pPT = ctx.enter_context(tc.tile_pool(name="pPT", bufs=1))
    pNUM = ctx.enter_context(tc.tile_pool(name="pNUM", bufs=1))
    pOUT = ctx.enter_context(tc.tile_pool(name="pOUT", bufs=2))
    pRB = ctx.enter_context(tc.tile_pool(name="pRB", bufs=2))
    pS = ctx.enter_context(tc.tile_pool(name="pS", bufs=2))
    pmm = ctx.enter_context(tc.tile_pool(name="pmm", bufs=4, space="PSUM"))
    paux = ctx.enter_context(tc.tile_pool(name="paux", bufs=2, space="PSUM"))
    ptiny = ctx.enter_context(tc.tile_pool(name="ptiny", bufs=2, space="PSUM"))

    # ---- image 0's x~ (bf16, pre-scaled by 2048) first: it gates everything.
    # Split across both HWDGE queues (sync + scalar).
    X16_0 = pX16.tile([P, NCT, HW], DT, name="X16_0", tag="X16")
    for ct in range(NCT):
        (nc.sync if ct % 2 == 0 else nc.scalar).dma_start(
            X16_0[:, ct, :], io["x16"][0, ct * P:(ct + 1) * P, :])

    def load_const(name, shape, dtype=F32):
        t = consts.tile(list(shape), dtype, name=f"c_{name}")
        nc.sync.dma_start(t[:], io[name][:])
        return t

    # all (P, *) vectors packed into ONE DMA
    cvec = load_const("cvec", (P, 5 * NCT + GPT))
    bq_sb = cvec[:, 0 * NCT:1 * NCT]     # 32*bq
    bk_sb = cvec[:, 1 * NCT:2 * NCT]     # 32*bk
    bo_sb = cvec[:, 2 * NCT:3 * NCT]     # 2048*bo
    gs_sb = cvec[:, 3 * NCT:4 * NCT]
    gb_sb = cvec[:, 4 * NCT:5 * NCT]
    gsel = cvec[:, 5 * NCT:5 * NCT + GPT]
    gselT = load_const("gselT", (GPT, P))

    # ---- fp8 weights (x32), one packed DMA per matrix: [P, NCT, C]
    w_sb = {}
    for wname in ("wqt", "wkt", "wvt", "wot"):
        t = consts.tile([P, NCT, C], F8, name=f"{wname}_p")
        nc.sync.dma_start(t[:], io[wname][:])
        w_sb[wname] = t

    # dual-fp8 LdWeights needs the k-tile stride 16B-aligned: pad to [P,2,16]
    # and slice one column.
    ones8_t = consts.tile([P, 2, 16], F8, name="ones8")
    nc.vector.memset(ones8_t[:], 1.0)
    ones8 = ones8_t[:, :, 0:1]
    ones_row = consts.tile([1, P], DT, name="ones_row")
    nc.vector.memset(ones_row[:], 1.0)
    zb = consts.tile([P, 1], F32, name="zb")
    nc.vector.memset(zb[:], 0.0)
    epsb = consts.tile([GPT, 1], F32, name="epsb")
    nc.vector.memset(epsb[:], EPS_T)

    # ---- per-image emission ----
    def new_img(i):
        return {"i": i}

    def emit_load16(im):
        i = im["i"]
        if i == 0:
            im["X16"] = X16_0
            return
        X16 = pX16.tile([P, NCT, HW], DT, name=f"X16_{i}", tag="X16")
        for ct in range(NCT):
            (nc.sync if ct % 2 == 0 else nc.scalar).dma_start(
                X16[:, ct, :], io["x16"][i, ct * P:(ct + 1) * P, :])
        im["X16"] = X16

    def emit_stats(im):
        i = im["i"]
        X16 = im["X16"]
        stats = pS.tile([P, 2 * NCT], F32, name=f"stats{i}", tag="stats")
        scratch = pS.tile([P, HW], DT, name=f"scr{i}", tag="scratch")
        for ct in range(NCT):
            nc.vector.tensor_reduce(stats[:, ct:ct + 1], X16[:, ct, :], AX.X, OP.add)
            nc.scalar.activation(scratch[:], X16[:, ct, :], AF.Square, bias=zb[:],
                                 accum_out=stats[:, NCT + ct:NCT + ct + 1])
        im["stats"] = stats

    def emit_norm(im, fast):
        # fast=True spreads hn tiles across ACT/DVE/Pool (image 0 gates the
        # first qkv); fast=False leaves them all on Pool.
        i = im["i"]
        X16, stats = im["X16"], im["stats"]
        with nc.named_scope(f"norm{i}"):
            gst = ptiny.tile([GPT, 2 * NCT], F32, name=f"gst{i}", tag="tiny")
            nc.tensor.matmul(gst[:], gsel[:], stats[:], start=True, stop=True)
            gm = pS.tile([GPT, 2 * NCT], F32, name=f"gm{i}", tag="gm")
            nc.vector.tensor_scalar_mul(gm[:], gst[:], 1.0 / (CPG * HW))
            sq = pS.tile([GPT, NCT], F32, name=f"sq{i}", tag="sq")
            nc.vector.tensor_mul(sq[:], gm[:, 0:NCT], gm[:, 0:NCT])
            var = pS.tile([GPT, NCT], F32, name=f"var{i}", tag="var")
            nc.vector.tensor_sub(var[:], gm[:, NCT:], sq[:])
            std = pS.tile([GPT, NCT], F32, name=f"std{i}", tag="std")
            nc.scalar.activation(std[:], var[:], AF.Sqrt, bias=epsb[:])
            gmr = pS.tile([GPT, 2 * NCT], F32, name=f"gmr{i}", tag="gmr")
            nc.vector.tensor_copy(gmr[:, 0:NCT], gm[:, 0:NCT])
            nc.vector.reciprocal(gmr[:, NCT:], std[:])
            pmr = ptiny.tile([P, 2 * NCT], F32, name=f"pmr{i}", tag="tiny")
            nc.tensor.matmul(pmr[:], gselT[:], gmr[:], start=True, stop=True)
            mr = pS.tile([P, 2 * NCT], F32, name=f"mr{i}", tag="mr")
            nc.vector.tensor_copy(mr[:], pmr[:])
            # a = rstd~*scale (cols NCT..), b = gn_bias - mean~*a (cols 0..NCT)
            ab = pS.tile([P, 2 * NCT], F32, name=f"ab{i}", tag="ab")
            tb = pS.tile([P, NCT], F32, name=f"tb{i}", tag="tb")
            for ct in range(NCT):
                a_col = ab[:, NCT + ct:NCT + ct + 1]
                nc.vector.tensor_mul(a_col, mr[:, NCT + ct:NCT + ct + 1], gs_sb[:, ct:ct + 1])
                nc.vector.tensor_mul(tb[:, ct:ct + 1], mr[:, ct:ct + 1], a_col)
                nc.vector.tensor_sub(ab[:, ct:ct + 1], gb_sb[:, ct:ct + 1], tb[:, ct:ct + 1])
            HN = pHN.tile([P, NCT, HW], F8, name=f"HN{i}", tag="HN")
            for ct in range(NCT):
                a_col = ab[:, NCT + ct:NCT + ct + 1]
                b_col = ab[:, ct:ct + 1]
                if fast and ct == 0:
                    nc.scalar.activation(HN[:, ct, :], X16[:, ct, :], AF.Identity,
                                         bias=b_col, scale=a_col)
                elif fast and ct == 1:
                    nc.vector.tensor_scalar(HN[:, ct, :], X16[:, ct, :],
                                            a_col, b_col, OP.mult, OP.add)
                else:
                    nc.gpsimd.tensor_scalar(HN[:, ct, :], X16[:, ct, :],
                                            a_col, b_col, OP.mult, OP.add)
            im["HN"] = HN

    def emit_qkv(im):
        i = im["i"]
        HN = im["HN"]
        with nc.named_scope(f"qkv{i}"):
            Q = pQ.tile([P, NCT, HW], F8, name=f"Q{i}", tag="Q")
            K = pK.tile([P, NCT, HW], F8, name=f"K{i}", tag="K")
            for wname, bias_sb, OT, ev in (("wqt", bq_sb, Q, "act"),
                                           ("wkt", bk_sb, K, "dve")):
                for ob in range(NCT):
                    ps = [pmm.tile([P, FC], F32, name=f"{wname}ps{i}_{ob}_{ic}", tag="mm")
                          for ic in range(NIC)]
                    for kp in range(NKP):
                        lhs = w_sb[wname][:, 2 * kp:2 * kp + 2, ob * P:(ob + 1) * P]
                        for ic in range(NIC):
                            _mm8(nc, ps[ic][:], lhs,
                                 HN[:, 2 * kp:2 * kp + 2, ic * FC:(ic + 1) * FC],
                                 start=(kp == 0), stop=(kp == NKP - 1))
                    for ic in range(NIC):
                        dst = OT[:, ob, ic * FC:(ic + 1) * FC]
                        if ev == "act":
                            nc.scalar.activation(dst, ps[ic][:], AF.Identity,
                                                 bias=bias_sb[:, ob:ob + 1])
                        else:
                            nc.vector.tensor_scalar_add(dst, ps[ic][:],
                                                        bias_sb[:, ob:ob + 1])
            VT = pVT.tile([P, NSB, C], F8, name=f"VT{i}", tag="VT")
            for sb in range(NSB):
                ps = pmm.tile([P, C], F32, name=f"vtps{i}_{sb}", tag="mm")
                for kp in range(NKP):
                    _mm8(nc, ps[:], HN[:, 2 * kp:2 * kp + 2, sb * P:(sb + 1) * P],
                         w_sb["wvt"][:, 2 * kp:2 * kp + 2, :],
                         start=(kp == 0), stop=(kp == NKP - 1))
                nc.scalar.copy(VT[:, sb, :], ps[:])
            im["Q"], im["K"], im["VT"] = Q, K, VT

    def emit_scores(im):
        i = im["i"]
        Q, K = im["Q"], im["K"]
        with nc.named_scope(f"scores{i}"):
            PT = pPT.tile([P, NSB, HW], F8, name=f"PT{i}", tag="PT")
            for jb in range(NSB):
                ps = [pmm.tile([P, FC], F32, name=f"sps{i}_{jb}_{ic}", tag="mm")
                      for ic in range(NIC)]
                for kp in range(NKP):
                    lhs = K[:, 2 * kp:2 * kp + 2, jb * P:(jb + 1) * P]
                    for ic in range(NIC):
                        _mm8(nc, ps[ic][:], lhs,
                             Q[:, 2 * kp:2 * kp + 2, ic * FC:(ic + 1) * FC],
                             start=(kp == 0), stop=(kp == NKP - 1))
                for ic in range(NIC):
                    nc.scalar.activation(PT[:, jb, ic * FC:(ic + 1) * FC], ps[ic][:],
                                         AF.Exp, bias=zb[:],
                                         scale=SM_SCALE / (SW * SW))
            im["PT"] = PT

    def emit_den(im):
        # den, recip, and the recip broadcast to all partitions
        i = im["i"]
        PT = im["PT"]
        with nc.named_scope(f"den{i}"):
            recip = pS.tile([1, HW], F32, name=f"recip{i}", tag="recip")
            recip_dt = pS.tile([1, HW], DT, name=f"recipdt{i}", tag="recipdt")
            for ic in range(NIC):
                den = paux.tile([1, FC], F32, name=f"den{i}_{ic}", tag="aux")
                for jp in range(NJP):
                    _mm8(nc, den[:], ones8[:],
                         PT[:, 2 * jp:2 * jp + 2, ic * FC:(ic + 1) * FC],
                         start=(jp == 0), stop=(jp == NJP - 1))
                sl = slice(ic * FC, (ic + 1) * FC)
                nc.vector.reciprocal(recip[:, sl], den[:])
                # SR folds the extra NUM scale so NUM lands at 64*num
                nc.vector.tensor_scalar_mul(recip_dt[:, sl], recip[:, sl], SR)
            recipb = pRB.tile([P, HW], F32, name=f"recipb{i}", tag="recipb")
            for ic in range(NIC):
                rb = paux.tile([P, FC], F32, name=f"rb{i}_{ic}", tag="aux")
                nc.tensor.matmul(rb[:], ones_row[:],
                                 recip_dt[:, ic * FC:(ic + 1) * FC],
                                 start=True, stop=True)
                nc.vector.tensor_copy(recipb[:, ic * FC:(ic + 1) * FC], rb[:])
            im["recipb"] = recipb

    def emit_attn_out(im):
        i = im["i"]
        X16, VT, PT, recipb = im["X16"], im["VT"], im["PT"], im["recipb"]
        with nc.named_scope(f"attnout{i}"):
            NUM = pNUM.tile([P, NCT, HW], F8, name=f"NUM{i}", tag="NUM")
            for cb in range(NCT):
                ps = [pmm.tile([P, FC], F32, name=f"nps{i}_{cb}_{ic}", tag="mm")
                      for ic in range(NIC)]
                for jp in range(NJP):
                    lhs = VT[:, 2 * jp:2 * jp + 2, cb * P:(cb + 1) * P]
                    for ic in range(NIC):
                        _mm8(nc, ps[ic][:], lhs,
                             PT[:, 2 * jp:2 * jp + 2, ic * FC:(ic + 1) * FC],
                             start=(jp == 0), stop=(jp == NJP - 1))
                for ic in range(NIC):
                    sl = slice(ic * FC, (ic + 1) * FC)
                    nc.vector.tensor_mul(NUM[:, cb, sl], ps[ic][:], recipb[:, sl])
            # proj (+2048*bo) + 2048*residual straight from PSUM, then store
            OUTT = pOUT.tile([P, NCT, HW], DT, name=f"OUT{i}", tag="OUT")
            for ob in range(NCT):
                ps = [pmm.tile([P, FC], F32, name=f"pps{i}_{ob}_{ic}", tag="mm")
                      for ic in range(NIC)]
                for kp in range(NKP):
                    lhs = w_sb["wot"][:, 2 * kp:2 * kp + 2, ob * P:(ob + 1) * P]
                    for ic in range(NIC):
                        _mm8(nc, ps[ic][:], lhs,
                             NUM[:, 2 * kp:2 * kp + 2, ic * FC:(ic + 1) * FC],
                             start=(kp == 0), stop=(kp == NKP - 1))
                for ic in range(NIC):
                    sl = slice(ic * FC, (ic + 1) * FC)
                    nc.vector.scalar_tensor_tensor(OUTT[:, ob, sl], ps[ic][:],
                                                   bo_sb[:, ob:ob + 1], X16[:, ob, sl],
                                                   OP.add, OP.add)
                    (nc.sync if ic == 0 else nc.scalar).dma_start(
                        io["out"][i, ob * P:(ob + 1) * P, sl], OUTT[:, ob, sl])

    ims = [new_img(i) for i in range(BPC)]
    a, b = ims
    emit_load16(a)
    emit_stats(a)
    emit_load16(b)
    emit_stats(b)
    emit_norm(a, fast=True)
    emit_norm(b, fast=False)
    emit_qkv(a)
    emit_scores(a)
    emit_den(a)
    emit_qkv(b)
    emit_attn_out(a)
    emit_scores(b)
    emit_den(b)
    emit_attn_out(b)


def _build():
    if "nc" in _CACHE:
        return _CACHE["nc"]
    nc = bacc.Bacc("TRN2", target_bir_lowering=False, debug=False, num_devices=NCORES)
    io = {}
    io["x16"] = nc.dram_tensor("x16", [BPC, C, HW], DT, kind="ExternalInput").ap()
    for wname in ("wqt", "wkt", "wvt", "wot"):
        io[wname] = nc.dram_tensor(wname, [P, NCT, C], F8, kind="ExternalInput").ap()
    io["cvec"] = nc.dram_tensor("cvec", [P, 5 * NCT + GPT], F32,
                                kind="ExternalInput").ap()
    io["gselT"] = nc.dram_tensor("gselT", [GPT, P], F32, kind="ExternalInput").ap()
    io["out"] = nc.dram_tensor("out", [BPC, C, HW], DT, kind="ExternalOutput").ap()

    with tile.TileContext(nc) as tc:
        with ExitStack() as ctx:
            _emit(ctx, tc, io)
    nc.compile()
    _CACHE["nc"] = nc
    return nc


def _col_layout(v, scale=1.0):
    # (C,) -> (P, NCT): column ct holds channels [ct*128, (ct+1)*128)
    return np.ascontiguousarray(
        (np.asarray(v, np.float32) * scale).reshape(NCT, P).T)


def _run(inputs, trace=False, **run_kwargs):
    x = np.asarray(inputs["x"], np.float32).reshape(B, C, HW)

    def _wpack(w):
        # w (c_out, c_in) -> [P, NCT, C] fp8 of 32*w.T
        wt = (np.asarray(w, np.float32).T * SW).astype(F8_NP)
        return np.ascontiguousarray(wt.reshape(NCT, P, C).transpose(1, 0, 2))

    wdt = {n: _wpack(inputs[s])
           for n, s in (("wqt", "wq"), ("wkt", "wk"), ("wvt", "wv"), ("wot", "wo"))}
    pidx = np.arange(P)
    gsel = (pidx[:, None] // CPG == np.arange(GPT)[None, :]).astype(np.float32)
    # bv never appears on-chip: sum_j P = den exactly, so it lands as wo @ bv
    bo_eff = (np.asarray(inputs["bo"], np.float32)
              + np.asarray(inputs["wo"], np.float32)
              @ np.asarray(inputs["bv"], np.float32))
    cvec = np.concatenate([_col_layout(inputs["bq"], SW),
                           _col_layout(inputs["bk"], SW),
                           _col_layout(bo_eff, SX),
                           _col_layout(inputs["gn_scale"]),
                           _col_layout(inputs["gn_bias"]), gsel], axis=1)
    common = {
        **wdt,
        "cvec": np.ascontiguousarray(cvec),
        "gselT": np.ascontiguousarray(gsel.T),
    }
    x16 = (x * SX).astype(DT_NP)   # 2048*x in bf16 (exact exponent shift)
    in_maps = [{"x16": np.ascontiguousarray(x16[m * BPC:(m + 1) * BPC]), **common}
               for m in range(NCORES)]
    nc = _build()
    res = run_bass_kernel_spmd(nc, in_maps, core_ids=list(range(NCORES)),
                               trace=trace, **run_kwargs)
    out = np.concatenate([r["out"] for r in res.results], axis=0)
    out = out.astype(np.float32) * (1.0 / SX)
    return out.reshape(B, C, H, W), res


def kernel(**inputs):
    out, _ = _run(inputs)
    return out


# revision 12
# speedup vs baseline: 1.5761x; 1.0137x over previous
"""AttnBlock (GroupNorm + single-head spatial self-attention + residual) on 8 TRN2 cores.

Sharding: data-parallel over batch — B=16 images, 2 per NeuronCore. Each core runs
an identical Bass/Tile program over its 2 images; no cross-core communication.

All matmuls run in fp8 (e4m3) with DoubleRow perf mode: each matmul contracts
K=256 (two 128-partition k-tiles) at 2 moving elements/cycle — 2x the bf16
tensor-engine throughput (measured 215ns steady-state issue cadence for a
[128,512]-out DR matmul = full 2.4GHz double-pumped). fp32 PSUM accumulation.

Scale management (e4m3 normals start at 2^-6, so small tensors must be
pre-scaled; all scales are powers of two and cancel exactly):
  - weights wq,wk,wv,wo are stored as fp8(32*w)         (w sigma ~0.02)
  - hn (GroupNorm output, sigma 1) is unscaled fp8
  - q,k carry x32; the exp() activation scale folds 1/(32*32) with C^-0.5
  - v carries x32; NUM = num_psum * (2/den) carries 64x (num sigma ~0.013)
  - proj psum carries 32*64 = 2048x; the residual x is pre-scaled by 2048
    host-side (exact in bf16), so the final eviction is a single
    scalar_tensor_tensor and the host divides the bf16 output by 2048.

GroupNorm stats come from one bn_stats+bn_aggr pass per channel tile (DVE)
on x~ = 2048*x in bf16; eps~ = 2048^2*eps keeps the pipeline scale-invariant
and the per-channel affine a,b come out in x~ units. Residual and output stay
bf16 (the attention branch is ~0.6% of the output norm; end-to-end error is
~2.4e-3, dominated by the bf16 residual/output quantization, an 8x margin to
the 2e-2 gate).

The v bias never appears on-chip: since sum_j P[j,i] = den[i] holds exactly for
the stored PT8, bv contributes wo @ bv to the output, folded into bo host-side.
The V eviction is then a pure PSUM->SBUF copy.

PSUM tiles are [128, 2, 512] fp32 (two banks); each DR matmul writes one
512-column bank, evictions process both banks in a single instruction.

Engine split (GPSIMD cannot access PSUM, so PSUM evictions live on ACT/DVE):
  ACT  : q eviction (+bias), V eviction (copy), exp->PT8, 1 hn tile
  DVE  : bn stats, k eviction, NUM eviction (*recip), OUT eviction (+resid),
         reciprocal_approx_fast chain, GroupNorm combine glue, 1 hn tile
  Pool : hn tiles, output-store DMA descriptors
"""

import numpy as np
import ml_dtypes
from contextlib import ExitStack

import concourse.bass as bass
import concourse.bacc as bacc
import concourse.tile as tile
import concourse.mybir as mybir
from concourse.bass_utils import run_bass_kernel_spmd

F32 = mybir.dt.float32
AF = mybir.ActivationFunctionType
OP = mybir.AluOpType
AX = mybir.AxisListType
PM = mybir.MatmulPerfMode

B, C, H, W = 16, 512, 32, 32
HW = H * W            # 1024
G = 32                # groupnorm groups
CPG = C // G          # 16 channels per group
EPS = 1e-5
NCORES = 8
BPC = B // NCORES     # 2 images per core
P = 128               # SBUF partitions
NCT = C // P          # 4 channel tiles
NKP = NCT // 2        # 2 k-tile pairs for DoubleRow over C
GPT = P // CPG        # 8 groups per channel tile
NSB = HW // P         # 8 spatial blocks of 128
NJP = NSB // 2        # 4 k-tile pairs for DoubleRow over HW
FC = 512              # matmul free-dim chunk (one PSUM bank of fp32)
NIC = HW // FC        # 2 chunks over the spatial free dim
SM_SCALE = float(C) ** -0.5

SW = 32.0             # weight scale (all four conv weights)
SR = 2.0              # extra scale folded into recip (NUM carries SW*SR=64)
SX = SW * SW * SR     # 2048: residual/output scale
EPS_T = SX * SX * EPS # eps for stats computed on 2048*x

DT = mybir.dt.bfloat16
DT_NP = ml_dtypes.bfloat16
F8 = mybir.dt.float8e4
F8_NP = ml_dtypes.float8_e4m3

_CACHE: dict = {}


def _mm8(nc, out, lhsT, rhs, start, stop):
    nc.tensor.matmul(out, lhsT, rhs, start=start, stop=stop,
                     perf_mode=PM.DoubleRow)


def _emit(ctx, tc, io):
    nc = tc.nc

    consts = ctx.enter_context(tc.tile_pool(name="consts", bufs=1))
    pX16 = ctx.enter_context(tc.tile_pool(name="pX16", bufs=2))
    pHN = ctx.enter_context(tc.tile_pool(name="pHN", bufs=2))
    pQ = ctx.enter_context(tc.tile_pool(name="pQ", bufs=1))
    pK = ctx.enter_context(tc.tile_pool(name="pK", bufs=1))
    pVT = ctx.enter_context(tc.tile_pool(name="pVT", bufs=2))
    pPT = ctx.enter_context(tc.tile_pool(name="pPT", bufs=1))
    pNUM = ctx.enter_context(tc.tile_pool(name="pNUM", bufs=1))
    pOUT = ctx.enter_context(tc.tile_pool(name="pOUT", bufs=2))
    pRB = ctx.enter_context(tc.tile_pool(name="pRB", bufs=2))
    pS = ctx.enter_context(tc.tile_pool(name="pS", bufs=2))
    # two PSUM pools of 2x [128,2,512] fp32 tiles (2 banks each) = 8 banks
    pmm = ctx.enter_context(tc.tile_pool(name="pmm", bufs=2, space="PSUM"))
    paux = ctx.enter_context(tc.tile_pool(name="paux", bufs=2, space="PSUM"))

    # ---- image 0's x~ (bf16, pre-scaled by 2048) first: it gates everything.
    # Host-packed [P, NCT*HW]; 4 descriptors split across sync+scalar queues.
    X16_0 = pX16.tile([P, NCT, NIC, FC], DT, name="X16_0", tag="X16")
    for ct in range(NCT):
        (nc.sync if ct % 2 == 0 else nc.scalar).dma_start(
            X16_0[:, ct], io["x16"][0, :, ct * HW:(ct + 1) * HW])

    def load_const(name, shape, dtype=F32):
        t = consts.tile(list(shape), dtype, name=f"c_{name}")
        nc.sync.dma_start(t[:], io[name][:])
        return t

    # all (P, *) vectors packed into ONE DMA
    cvec = load_const("cvec", (P, 5 * NCT + GPT))
    bq_sb = cvec[:, 0 * NCT:1 * NCT]     # 32*bq
    bk_sb = cvec[:, 1 * NCT:2 * NCT]     # 32*bk
    bo_sb = cvec[:, 2 * NCT:3 * NCT]     # 2048*(bo + wo@bv)
    gs_sb = cvec[:, 3 * NCT:4 * NCT]
    gb_sb = cvec[:, 4 * NCT:5 * NCT]
    gsel = cvec[:, 5 * NCT:5 * NCT + GPT]
    gselT = load_const("gselT", (GPT, P))

    # ---- fp8 weights (x32), one packed DMA per matrix: [P, NCT, C]
    w_sb = {}
    for wname in ("wqt", "wkt", "wvt", "wot"):
        t = consts.tile([P, NCT, C], F8, name=f"{wname}_p")
        nc.scalar.dma_start(t[:], io[wname][:])
        w_sb[wname] = t

    # dual-fp8 LdWeights needs the k-tile stride 16B-aligned: pad to [P,2,16]
    ones8_t = consts.tile([P, 2, 16], F8, name="ones8")
    nc.vector.memset(ones8_t[:], 1.0)
    ones8 = ones8_t[:, :, 0:1]
    ones_row = consts.tile([1, P], DT, name="ones_row")
    nc.vector.memset(ones_row[:], 1.0)
    zb = consts.tile([P, 1], F32, name="zb")
    nc.vector.memset(zb[:], 0.0)
    epsb = consts.tile([GPT, 1], F32, name="epsb")
    nc.vector.memset(epsb[:], EPS_T)

    # ---- per-image emission ----
    def new_img(i):
        return {"i": i}

    def emit_load16(im):
        i = im["i"]
        if i == 0:
            im["X16"] = X16_0
            return
        X16 = pX16.tile([P, NCT, NIC, FC], DT, name=f"X16_{i}", tag="X16")
        nc.sync.dma_start(X16[:, 0:2], io["x16"][i, :, 0:2 * HW])
        nc.scalar.dma_start(X16[:, 2:4], io["x16"][i, :, 2 * HW:4 * HW])
        im["X16"] = X16

    def emit_stats(im):
        # one bn_stats+bn_aggr pass per channel tile -> per-channel mean/var
        i = im["i"]
        X16 = im["X16"]
        bnst = pS.tile([P, NCT, NIC, 6], F32, name=f"bnst{i}", tag="bnst")
        st2 = pS.tile([P, NCT, 2], F32, name=f"st2_{i}", tag="st2")
        for ct in range(NCT):
            for ic in range(NIC):
                nc.vector.bn_stats(bnst[:, ct, ic], X16[:, ct, ic])
            nc.vector.bn_aggr(st2[:, ct], bnst[:, ct])
        # gsel-matmul input: cols 0..NCT = mean, NCT.. = E[x^2] = var + mean^2
        stats = pS.tile([P, 2 * NCT], F32, name=f"stats{i}", tag="stats")
        nc.vector.tensor_copy(stats[:, 0:NCT], st2[:, :, 0])
        msq = pS.tile([P, NCT], F32, name=f"msq{i}", tag="msq")
        nc.vector.tensor_mul(msq[:], st2[:, :, 0], st2[:, :, 0])
        nc.vector.tensor_add(stats[:, NCT:], st2[:, :, 1], msq[:])
        im["stats"] = stats

    def emit_norm(im, fast):
        # fast=True spreads hn tiles across ACT/DVE/Pool (image 0 gates the
        # first qkv); fast=False leaves them all on Pool.
        i = im["i"]
        X16, stats = im["X16"], im["stats"]
        with nc.named_scope(f"norm{i}"):
            gst = paux.tile([GPT, 2 * NCT], F32, name=f"gst{i}", tag="aux")
            nc.tensor.matmul(gst[:], gsel[:], stats[:], start=True, stop=True)
            gm = pS.tile([GPT, 2 * NCT], F32, name=f"gm{i}", tag="gm")
            nc.vector.tensor_scalar_mul(gm[:], gst[:], 1.0 / CPG)
            sq = pS.tile([GPT, NCT], F32, name=f"sq{i}", tag="sq")
            nc.vector.tensor_mul(sq[:], gm[:, 0:NCT], gm[:, 0:NCT])
            var = pS.tile([GPT, NCT], F32, name=f"var{i}", tag="var")
            nc.vector.tensor_sub(var[:], gm[:, NCT:], sq[:])
            std = pS.tile([GPT, NCT], F32, name=f"std{i}", tag="std")
            nc.scalar.activation(std[:], var[:], AF.Sqrt, bias=epsb[:])
            gmr = pS.tile([GPT, 2 * NCT], F32, name=f"gmr{i}", tag="gmr")
            nc.vector.tensor_copy(gmr[:, 0:NCT], gm[:, 0:NCT])
            nc.vector.reciprocal_approx_fast(gmr[:, NCT:], std[:])
            pmr = paux.tile([P, 2 * NCT], F32, name=f"pmr{i}", tag="aux")
            nc.tensor.matmul(pmr[:], gselT[:], gmr[:], start=True, stop=True)
            mr = pS.tile([P, 2 * NCT], F32, name=f"mr{i}", tag="mr")
            nc.vector.tensor_copy(mr[:], pmr[:])
            # a = rstd~*scale (cols NCT..), b = gn_bias - mean~*a (cols 0..NCT)
            ab = pS.tile([P, 2 * NCT], F32, name=f"ab{i}", tag="ab")
            tb = pS.tile([P, NCT], F32, name=f"tb{i}", tag="tb")
            for ct in range(NCT):
                a_col = ab[:, NCT + ct:NCT + ct + 1]
                nc.vector.tensor_mul(a_col, mr[:, NCT + ct:NCT + ct + 1], gs_sb[:, ct:ct + 1])
                nc.vector.tensor_mul(tb[:, ct:ct + 1], mr[:, ct:ct + 1], a_col)
                nc.vector.tensor_sub(ab[:, ct:ct + 1], gb_sb[:, ct:ct + 1], tb[:, ct:ct + 1])
            HN = pHN.tile([P, NCT, NIC, FC], F8, name=f"HN{i}", tag="HN")
            for ct in range(NCT):
                a_col = ab[:, NCT + ct:NCT + ct + 1]
                b_col = ab[:, ct:ct + 1]
                if fast and ct == 0:
                    nc.scalar.activation(HN[:, ct], X16[:, ct], AF.Identity,
                                         bias=b_col, scale=a_col)
                elif fast and ct == 1:
                    nc.vector.tensor_scalar(HN[:, ct], X16[:, ct],
                                            a_col, b_col, OP.mult, OP.add)
                else:
                    nc.gpsimd.tensor_scalar(HN[:, ct], X16[:, ct],
                                            a_col, b_col, OP.mult, OP.add)
            im["HN"] = HN

    def emit_qkv(im):
        i = im["i"]
        HN = im["HN"]
        with nc.named_scope(f"qkv{i}"):
            Q = pQ.tile([P, NCT, NIC, FC], F8, name=f"Q{i}", tag="Q")
            K = pK.tile([P, NCT, NIC, FC], F8, name=f"K{i}", tag="K")
            for wname, bias_sb, OT, ev in (("wqt", bq_sb, Q, "act"),
                                           ("wkt", bk_sb, K, "dve")):
                for ob in range(NCT):
                    ps = pmm.tile([P, NIC, FC], F32, name=f"{wname}ps{i}_{ob}", tag="mm")
                    for kp in range(NKP):
                        lhs = w_sb[wname][:, 2 * kp:2 * kp + 2, ob * P:(ob + 1) * P]
                        for ic in range(NIC):
                            _mm8(nc, ps[:, ic], lhs,
                                 HN[:, 2 * kp:2 * kp + 2, ic],
                                 start=(kp == 0), stop=(kp == NKP - 1))
                    if ev == "act":
                        nc.scalar.activation(OT[:, ob], ps[:], AF.Identity,
                                             bias=bias_sb[:, ob:ob + 1])
                    else:
                        nc.vector.tensor_scalar_add(OT[:, ob], ps[:],
                                                    bias_sb[:, ob:ob + 1])
            VT = pVT.tile([P, NSB, C], F8, name=f"VT{i}", tag="VT")
            for sp in range(NSB // 2):
                ps = pmm.tile([P, NIC, FC], F32, name=f"vtps{i}_{sp}", tag="mm")
                for h in range(2):
                    sb = 2 * sp + h
                    for kp in range(NKP):
                        _mm8(nc, ps[:, h],
                             HN[:, 2 * kp:2 * kp + 2, sb // NJP,
                                (sb % NJP) * P:(sb % NJP + 1) * P],
                             w_sb["wvt"][:, 2 * kp:2 * kp + 2, :],
                             start=(kp == 0), stop=(kp == NKP - 1))
                nc.scalar.copy(VT[:, 2 * sp:2 * sp + 2, :], ps[:])
            im["Q"], im["K"], im["VT"] = Q, K, VT

    def emit_scores(im):
        i = im["i"]
        Q, K = im["Q"], im["K"]
        with nc.named_scope(f"scores{i}"):
            PT = pPT.tile([P, NSB, NIC, FC], F8, name=f"PT{i}", tag="PT")
            for jb in range(NSB):
                ps = pmm.tile([P, NIC, FC], F32, name=f"sps{i}_{jb}", tag="mm")
                for kp in range(NKP):
                    lhs = K[:, 2 * kp:2 * kp + 2, jb // NJP,
                            (jb % NJP) * P:(jb % NJP + 1) * P]
                    for ic in range(NIC):
                        _mm8(nc, ps[:, ic], lhs,
                             Q[:, 2 * kp:2 * kp + 2, ic],
                             start=(kp == 0), stop=(kp == NKP - 1))
                nc.scalar.activation(PT[:, jb], ps[:], AF.Exp, bias=zb[:],
                                     scale=SM_SCALE / (SW * SW))
            im["PT"] = PT

    def emit_den(im):
        # den + reciprocal; the broadcast to all partitions is emitted later
        # (emit_attn_out) so the DVE chain hides under PE matmul work.
        i = im["i"]
        PT = im["PT"]
        with nc.named_scope(f"den{i}"):
            recip_dt = pS.tile([1, NIC, FC], DT, name=f"recipdt{i}", tag="recipdt")
            den = paux.tile([1, NIC, FC], F32, name=f"den{i}", tag="aux")
            for ic in range(NIC):
                for jp in range(NJP):
                    _mm8(nc, den[:, ic], ones8[:],
                         PT[:, 2 * jp:2 * jp + 2, ic],
                         start=(jp == 0), stop=(jp == NJP - 1))
            recip = pS.tile([1, NIC, FC], F32, name=f"recip{i}", tag="recip")
            nc.vector.reciprocal_approx_fast(recip[:], den[:])
            # SR folds the extra NUM scale so NUM lands at 64*num
            nc.vector.tensor_scalar_mul(recip_dt[:], recip[:], SR)
            im["recip_dt"] = recip_dt

    def emit_attn_out(im):
        i = im["i"]
        X16, VT, PT = im["X16"], im["VT"], im["PT"]
        with nc.named_scope(f"attnout{i}"):
            NUM = pNUM.tile([P, NCT, NIC, FC], F8, name=f"NUM{i}", tag="NUM")
            recipb = pRB.tile([P, NIC, FC], F32, name=f"recipb{i}", tag="recipb")
            pss = {}
            for cb in range(NCT):
                ps = pmm.tile([P, NIC, FC], F32, name=f"nps{i}_{cb}", tag="mm")
                pss[cb] = ps
                for jp in range(NJP):
                    lhs = VT[:, 2 * jp:2 * jp + 2, cb * P:(cb + 1) * P]
                    for ic in range(NIC):
                        _mm8(nc, ps[:, ic], lhs,
                             PT[:, 2 * jp:2 * jp + 2, ic],
                             start=(jp == 0), stop=(jp == NJP - 1))
                if cb == 0:
                    # recip broadcast: hidden under cb0's matmuls
                    rb = paux.tile([P, NIC, FC], F32, name=f"rb{i}", tag="aux")
                    for ic in range(NIC):
                        nc.tensor.matmul(rb[:, ic], ones_row[:],
                                         im["recip_dt"][:, ic],
                                         start=True, stop=True)
                    nc.vector.tensor_copy(recipb[:], rb[:])
                else:
                    pcb = cb - 1
                    nc.vector.tensor_mul(NUM[:, pcb], pss.pop(pcb)[:], recipb[:])
            nc.vector.tensor_mul(NUM[:, NCT - 1], pss.pop(NCT - 1)[:], recipb[:])
            # proj (+2048*(bo+wo@bv)) + 2048*residual from PSUM, then store
            OUTT = pOUT.tile([P, NCT, NIC, FC], DT, name=f"OUT{i}", tag="OUT")
            for ob in range(NCT):
                ps = pmm.tile([P, NIC, FC], F32, name=f"pps{i}_{ob}", tag="mm")
                for kp in range(NKP):
                    lhs = w_sb["wot"][:, 2 * kp:2 * kp + 2, ob * P:(ob + 1) * P]
                    for ic in range(NIC):
                        _mm8(nc, ps[:, ic], lhs,
                             NUM[:, 2 * kp:2 * kp + 2, ic],
                             start=(kp == 0), stop=(kp == NKP - 1))
                nc.vector.scalar_tensor_tensor(OUTT[:, ob], ps[:],
                                               bo_sb[:, ob:ob + 1], X16[:, ob],
                                               OP.add, OP.add)
                nc.gpsimd.dma_start(io["out"][i, :, ob * HW:(ob + 1) * HW],
                                    OUTT[:, ob])

    ims = [new_img(i) for i in range(BPC)]
    a, b = ims
    emit_load16(a)
    emit_stats(a)
    emit_load16(b)
    emit_stats(b)
    emit_norm(a, fast=True)
    emit_norm(b, fast=False)
    emit_qkv(a)
    emit_scores(a)
    emit_den(a)
    emit_qkv(b)
    emit_attn_out(a)
    emit_scores(b)
    emit_den(b)
    emit_attn_out(b)


def _build():
    if "nc" in _CACHE:
        return _CACHE["nc"]
    nc = bacc.Bacc("TRN2", target_bir_lowering=False, debug=False, num_devices=NCORES)
    io = {}
    io["x16"] = nc.dram_tensor("x16", [BPC, P, NCT * HW], DT, kind="ExternalInput").ap()
    for wname in ("wqt", "wkt", "wvt", "wot"):
        io[wname] = nc.dram_tensor(wname, [P, NCT, C], F8, kind="ExternalInput").ap()
    io["cvec"] = nc.dram_tensor("cvec", [P, 5 * NCT + GPT], F32,
                                kind="ExternalInput").ap()
    io["gselT"] = nc.dram_tensor("gselT", [GPT, P], F32, kind="ExternalInput").ap()
    io["out"] = nc.dram_tensor("out", [BPC, P, NCT * HW], DT, kind="ExternalOutput").ap()

    with tile.TileContext(nc) as tc:
        with ExitStack() as ctx:
            _emit(ctx, tc, io)
    nc.compile()
    _CACHE["nc"] = nc
    return nc


def _col_layout(v, scale=1.0):
    # (C,) -> (P, NCT): column ct holds channels [ct*128, (ct+1)*128)
    return np.ascontiguousarray(
        (np.asarray(v, np.float32) * scale).reshape(NCT, P).T)


def _run(inputs, trace=False, **run_kwargs):
    x = np.asarray(inputs["x"], np.float32).reshape(B, C, HW)

    def _wpack(w):
        # w (c_out, c_in) -> [P, NCT, C] fp8 of 32*w.T
        wt = (np.asarray(w, np.float32).T * SW).astype(F8_NP)
        return np.ascontiguousarray(wt.reshape(NCT, P, C).transpose(1, 0, 2))

    wdt = {n: _wpack(inputs[s])
           for n, s in (("wqt", "wq"), ("wkt", "wk"), ("wvt", "wv"), ("wot", "wo"))}
    pidx = np.arange(P)
    gsel = (pidx[:, None] // CPG == np.arange(GPT)[None, :]).astype(np.float32)
    # bv never appears on-chip: sum_j P = den exactly, so it lands as wo @ bv
    bo_eff = (np.asarray(inputs["bo"], np.float32)
              + np.asarray(inputs["wo"], np.float32)
              @ np.asarray(inputs["bv"], np.float32))
    cvec = np.concatenate([_col_layout(inputs["bq"], SW),
                           _col_layout(inputs["bk"], SW),
                           _col_layout(bo_eff, SX),
                           _col_layout(inputs["gn_scale"]),
                           _col_layout(inputs["gn_bias"]), gsel], axis=1)
    common = {
        **wdt,
        "cvec": np.ascontiguousarray(cvec),
        "gselT": np.ascontiguousarray(gsel.T),
    }
    # 2048*x in bf16 (exact exponent shift), packed [B, P, NCT*HW]
    x16 = (x * SX).astype(DT_NP)
    x16p = x16.reshape(B, NCT, P, HW).transpose(0, 2, 1, 3).reshape(B, P, NCT * HW)
    in_maps = [{"x16": np.ascontiguousarray(x16p[m * BPC:(m + 1) * BPC]), **common}
               for m in range(NCORES)]
    nc = _build()
    res = run_bass_kernel_spmd(nc, in_maps, core_ids=list(range(NCORES)),
                               trace=trace, **run_kwargs)
    out = np.concatenate([r["out"] for r in res.results], axis=0)
    # [B, P, NCT*HW] -> [B, C, HW], undo the 2048x scale
    out = out.reshape(B, P, NCT, HW).transpose(0, 2, 1, 3).reshape(B, C, HW)
    out = out.astype(np.float32) * (1.0 / SX)
    return out.reshape(B, C, H, W), res


def kernel(**inputs):
    out, _ = _run(inputs)
    return out


# revision 21
# speedup vs baseline: 1.6585x; 1.0522x over previous
"""AttnBlock (GroupNorm + single-head spatial self-attention + residual) on 8 TRN2 cores.

Sharding: data-parallel over batch — B=16 images, 2 per NeuronCore. Each core runs
an identical Bass/Tile program over its 2 images; no cross-core communication.

All matmuls run in fp8 (e4m3) with DoubleRow perf mode: each matmul contracts
K=256 (two 128-partition k-tiles) at 2 moving elements/cycle — 2x the bf16
tensor-engine throughput (measured 215ns steady-state issue cadence for a
[128,512]-out DR matmul = full 2.4GHz double-pumped). fp32 PSUM accumulation.

Scale management (e4m3 normals start at 2^-6, so small tensors must be
pre-scaled; all scales are powers of two and cancel exactly):
  - weights wq,wk,wv,wo are stored as fp8(32*w)         (w sigma ~0.02)
  - hn (GroupNorm output, sigma 1) is unscaled fp8
  - q,k carry x32; the exp() activation scale folds 1/(32*32) with C^-0.5
  - v carries x32; NUM = num_psum * (2/den) carries 64x (num sigma ~0.013)
  - proj psum carries 32*64 = 2048x; the residual x is pre-scaled by 2048
    host-side (exact in bf16), so the final eviction is a single
    scalar_tensor_tensor and the host divides the bf16 output by 2048.

GroupNorm stats come from one bn_stats+bn_aggr pass per channel tile (DVE)
on x~ = 2048*x in bf16; eps~ = 2048^2*eps keeps the pipeline scale-invariant
and the per-channel affine a,b come out in x~ units. Residual and output stay
bf16 (the attention branch is ~0.6% of the output norm; end-to-end error is
~2.4e-3, dominated by the bf16 residual/output quantization, an 8x margin to
the 2e-2 gate).

The v bias never appears on-chip: since sum_j P[j,i] = den[i] holds exactly for
the stored PT8, bv contributes wo @ bv to the output, folded into bo host-side.
The V eviction is then a pure PSUM->SBUF copy.

PSUM tiles are [128, 2, 512] fp32 (two banks); each DR matmul writes one
512-column bank, evictions process both banks in a single instruction.

Engine split (GPSIMD cannot access PSUM, so PSUM evictions live on ACT/DVE):
  ACT  : q eviction (+bias), V eviction (copy), exp->PT8, 1 hn tile
  DVE  : bn stats, k eviction, NUM eviction (*recip), OUT eviction (+resid),
         reciprocal_approx_fast chain, GroupNorm combine glue, 1 hn tile
  Pool : hn tiles, output-store DMA descriptors
"""

import numpy as np
import ml_dtypes
from contextlib import ExitStack

import concourse.bass as bass
import concourse.bacc as bacc
import concourse.tile as tile
import concourse.mybir as mybir
from concourse.bass_utils import run_bass_kernel_spmd

F32 = mybir.dt.float32
AF = mybir.ActivationFunctionType
OP = mybir.AluOpType
AX = mybir.AxisListType
PM = mybir.MatmulPerfMode

B, C, H, W = 16, 512, 32, 32
HW = H * W            # 1024
G = 32                # groupnorm groups
CPG = C // G          # 16 channels per group
EPS = 1e-5
NCORES = 8
BPC = B // NCORES     # 2 images per core
P = 128               # SBUF partitions
NCT = C // P          # 4 channel tiles
NKP = NCT // 2        # 2 k-tile pairs for DoubleRow over C
GPT = P // CPG        # 8 groups per channel tile
NSB = HW // P         # 8 spatial blocks of 128
NJP = NSB // 2        # 4 k-tile pairs for DoubleRow over HW
FC = 512              # matmul free-dim chunk (one PSUM bank of fp32)
NIC = HW // FC        # 2 chunks over the spatial free dim
SM_SCALE = float(C) ** -0.5

SW = 32.0             # weight scale (all four conv weights)
SR = 2.0              # extra scale folded into recip (NUM carries SW*SR=64)
SX = SW * SW * SR     # 2048: residual/output scale
EPS_T = SX * SX * EPS # eps for stats computed on 2048*x

DT = mybir.dt.bfloat16
DT_NP = ml_dtypes.bfloat16
F8 = mybir.dt.float8e4
F8_NP = ml_dtypes.float8_e4m3

_CACHE: dict = {}


def _mm8(nc, out, lhsT, rhs, start, stop):
    nc.tensor.matmul(out, lhsT, rhs, start=start, stop=stop,
                     perf_mode=PM.DoubleRow)


def _emit(ctx, tc, io):
    nc = tc.nc

    consts = ctx.enter_context(tc.tile_pool(name="consts", bufs=1))
    pX16 = ctx.enter_context(tc.tile_pool(name="pX16", bufs=2))
    pHN = ctx.enter_context(tc.tile_pool(name="pHN", bufs=2))
    pQ = ctx.enter_context(tc.tile_pool(name="pQ", bufs=1))
    pK = ctx.enter_context(tc.tile_pool(name="pK", bufs=1))
    pVT = ctx.enter_context(tc.tile_pool(name="pVT", bufs=2))
    # bufs=2: exp(b) writes PT1 while num(a) (emitted after scores(b)) still
    # reads PT0 — one buffer would WAR-deadlock against the pmm rotation
    pPT = ctx.enter_context(tc.tile_pool(name="pPT", bufs=2))
    pNUM = ctx.enter_context(tc.tile_pool(name="pNUM", bufs=1))
    pOUT = ctx.enter_context(tc.tile_pool(name="pOUT", bufs=2))
    pRB = ctx.enter_context(tc.tile_pool(name="pRB", bufs=2))
    pS = ctx.enter_context(tc.tile_pool(name="pS", bufs=2))
    # PSUM: pmm 2x[128,2,512] (4 banks) + den (2) + rb/gn (2) = 8 banks
    pmm = ctx.enter_context(tc.tile_pool(name="pmm", bufs=2, space="PSUM"))
    pden = ctx.enter_context(tc.tile_pool(name="pden", bufs=1, space="PSUM"))
    prb = ctx.enter_context(tc.tile_pool(name="prb", bufs=1, space="PSUM"))

    # ---- tiny consts first: the GroupNorm matmuls need gsel/gselT, so they
    # must not queue behind the bulk x16 transfers.
    def load_const(name, shape, dtype=F32):
        t = consts.tile(list(shape), dtype, name=f"c_{name}")
        nc.sync.dma_start(t[:], io[name][:])
        return t

    # all (P, *) vectors packed into ONE DMA
    cvec = load_const("cvec", (P, 5 * NCT + GPT))
    bq_sb = cvec[:, 0 * NCT:1 * NCT]     # 32*bq
    bk_sb = cvec[:, 1 * NCT:2 * NCT]     # 32*bk
    bo_sb = cvec[:, 2 * NCT:3 * NCT]     # 2048*(bo + wo@bv)
    gs_sb = cvec[:, 3 * NCT:4 * NCT]
    gb_sb = cvec[:, 4 * NCT:5 * NCT]
    gsel = cvec[:, 5 * NCT:5 * NCT + GPT]
    gselT = load_const("gselT", (GPT, P))

    # ---- image 0's x~ (bf16, pre-scaled by 2048): it gates the pipeline.
    # Host-packed [P, NCT*HW]; 4 descriptors split across sync+scalar queues.
    X16_0 = pX16.tile([P, NCT, NIC, FC], DT, name="X16_0", tag="X16")
    for ct in range(NCT):
        (nc.sync if ct % 2 == 0 else nc.scalar).dma_start(
            X16_0[:, ct], io["x16"][0, :, ct * HW:(ct + 1) * HW])

    # ---- fp8 weights (x32), one packed DMA per matrix: [P, NCT, C]
    w_sb = {}
    for wname in ("wqt", "wkt", "wvt", "wot"):
        t = consts.tile([P, NCT, C], F8, name=f"{wname}_p")
        nc.scalar.dma_start(t[:], io[wname][:])
        w_sb[wname] = t

    # dual-fp8 LdWeights needs the k-tile stride 16B-aligned: pad to [P,2,16]
    ones8_t = consts.tile([P, 2, 16], F8, name="ones8")
    nc.vector.memset(ones8_t[:], 1.0)
    ones8 = ones8_t[:, :, 0:1]
    ones_row = consts.tile([1, P], DT, name="ones_row")
    nc.vector.memset(ones_row[:], 1.0)
    zb = consts.tile([P, 1], F32, name="zb")
    nc.vector.memset(zb[:], 0.0)
    epsb = consts.tile([GPT, 1], F32, name="epsb")
    nc.vector.memset(epsb[:], EPS_T)

    # ---- per-image emission ----
    def new_img(i):
        return {"i": i}

    def emit_load16(im):
        i = im["i"]
        if i == 0:
            im["X16"] = X16_0
            return
        X16 = pX16.tile([P, NCT, NIC, FC], DT, name=f"X16_{i}", tag="X16")
        nc.sync.dma_start(X16[:, 0:2], io["x16"][i, :, 0:2 * HW])
        nc.scalar.dma_start(X16[:, 2:4], io["x16"][i, :, 2 * HW:4 * HW])
        im["X16"] = X16

    def emit_stats(im):
        # one bn_stats+bn_aggr pass per channel tile -> per-channel mean/var
        i = im["i"]
        X16 = im["X16"]
        bnst = pS.tile([P, NCT, NIC, 6], F32, name=f"bnst{i}", tag="bnst")
        st2 = pS.tile([P, NCT, 2], F32, name=f"st2_{i}", tag="st2")
        for ct in range(NCT):
            for ic in range(NIC):
                nc.vector.bn_stats(bnst[:, ct, ic], X16[:, ct, ic])
            nc.vector.bn_aggr(st2[:, ct], bnst[:, ct])
        # gsel-matmul input: cols 0..NCT = mean, NCT.. = E[x^2] = var + mean^2
        stats = pS.tile([P, 2 * NCT], F32, name=f"stats{i}", tag="stats")
        nc.vector.tensor_copy(stats[:, 0:NCT], st2[:, :, 0])
        msq = pS.tile([P, NCT], F32, name=f"msq{i}", tag="msq")
        nc.vector.tensor_mul(msq[:], st2[:, :, 0], st2[:, :, 0])
        nc.vector.tensor_add(stats[:, NCT:], st2[:, :, 1], msq[:])
        im["stats"] = stats

    def emit_norm(im, fast):
        # fast=True spreads hn tiles across ACT/DVE/Pool (image 0 gates the
        # first qkv); fast=False leaves them all on Pool.
        i = im["i"]
        X16, stats = im["X16"], im["stats"]
        with nc.named_scope(f"norm{i}"):
            gst = prb.tile([GPT, 2 * NCT], F32, name=f"gst{i}", tag="aux")
            nc.tensor.matmul(gst[:], gsel[:], stats[:], start=True, stop=True)
            gm = pS.tile([GPT, 2 * NCT], F32, name=f"gm{i}", tag="gm")
            nc.vector.tensor_scalar_mul(gm[:], gst[:], 1.0 / CPG)
            sq = pS.tile([GPT, NCT], F32, name=f"sq{i}", tag="sq")
            nc.vector.tensor_mul(sq[:], gm[:, 0:NCT], gm[:, 0:NCT])
            var = pS.tile([GPT, NCT], F32, name=f"var{i}", tag="var")
            nc.vector.tensor_sub(var[:], gm[:, NCT:], sq[:])
            std = pS.tile([GPT, NCT], F32, name=f"std{i}", tag="std")
            nc.scalar.activation(std[:], var[:], AF.Sqrt, bias=epsb[:])
            gmr = pS.tile([GPT, 2 * NCT], F32, name=f"gmr{i}", tag="gmr")
            nc.vector.tensor_copy(gmr[:, 0:NCT], gm[:, 0:NCT])
            nc.vector.reciprocal_approx_fast(gmr[:, NCT:], std[:])
            pmr = prb.tile([P, 2 * NCT], F32, name=f"pmr{i}", tag="aux")
            nc.tensor.matmul(pmr[:], gselT[:], gmr[:], start=True, stop=True)
            mr = pS.tile([P, 2 * NCT], F32, name=f"mr{i}", tag="mr")
            nc.vector.tensor_copy(mr[:], pmr[:])
            # a = rstd~*scale (cols NCT..), b = gn_bias - mean~*a (cols 0..NCT)
            ab = pS.tile([P, 2 * NCT], F32, name=f"ab{i}", tag="ab")
            tb = pS.tile([P, NCT], F32, name=f"tb{i}", tag="tb")
            for ct in range(NCT):
                a_col = ab[:, NCT + ct:NCT + ct + 1]
                nc.vector.tensor_mul(a_col, mr[:, NCT + ct:NCT + ct + 1], gs_sb[:, ct:ct + 1])
                nc.vector.tensor_mul(tb[:, ct:ct + 1], mr[:, ct:ct + 1], a_col)
                nc.vector.tensor_sub(ab[:, ct:ct + 1], gb_sb[:, ct:ct + 1], tb[:, ct:ct + 1])
            HN = pHN.tile([P, NCT, NIC, FC], F8, name=f"HN{i}", tag="HN")
            kinds = ("act", "dve", "pool", "act") if fast else ("pool",) * NCT
            for ct in range(NCT):
                a_col = ab[:, NCT + ct:NCT + ct + 1]
                b_col = ab[:, ct:ct + 1]
                if kinds[ct] == "act":
                    nc.scalar.activation(HN[:, ct], X16[:, ct], AF.Identity,
                                         bias=b_col, scale=a_col)
                elif kinds[ct] == "dve":
                    nc.vector.tensor_scalar(HN[:, ct], X16[:, ct],
                                            a_col, b_col, OP.mult, OP.add)
                else:
                    nc.gpsimd.tensor_scalar(HN[:, ct], X16[:, ct],
                                            a_col, b_col, OP.mult, OP.add)
            im["HN"] = HN

    def emit_qkv(im):
        i = im["i"]
        HN = im["HN"]
        with nc.named_scope(f"qkv{i}"):
            Q = pQ.tile([P, NCT, NIC, FC], F8, name=f"Q{i}", tag="Q")
            K = pK.tile([P, NCT, NIC, FC], F8, name=f"K{i}", tag="K")
            for wname, bias_sb, OT, ev in (("wqt", bq_sb, Q, "act"),
                                           ("wkt", bk_sb, K, "dve")):
                for ob in range(NCT):
                    ps = pmm.tile([P, NIC, FC], F32, name=f"{wname}ps{i}_{ob}", tag="mm")
                    for kp in range(NKP):
                        lhs = w_sb[wname][:, 2 * kp:2 * kp + 2, ob * P:(ob + 1) * P]
                        for ic in range(NIC):
                            _mm8(nc, ps[:, ic], lhs,
                                 HN[:, 2 * kp:2 * kp + 2, ic],
                                 start=(kp == 0), stop=(kp == NKP - 1))
                    if ev == "act":
                        nc.scalar.activation(OT[:, ob], ps[:], AF.Identity,
                                             bias=bias_sb[:, ob:ob + 1])
                    else:
                        nc.vector.tensor_scalar_add(OT[:, ob], ps[:],
                                                    bias_sb[:, ob:ob + 1])
            VT = pVT.tile([P, NSB, C], F8, name=f"VT{i}", tag="VT")
            for sp in range(NSB // 2):
                ps = pmm.tile([P, NIC, FC], F32, name=f"vtps{i}_{sp}", tag="mm")
                for h in range(2):
                    sb = 2 * sp + h
                    for kp in range(NKP):
                        _mm8(nc, ps[:, h],
                             HN[:, 2 * kp:2 * kp + 2, sb // NJP,
                                (sb % NJP) * P:(sb % NJP + 1) * P],
                             w_sb["wvt"][:, 2 * kp:2 * kp + 2, :],
                             start=(kp == 0), stop=(kp == NKP - 1))
                nc.scalar.copy(VT[:, 2 * sp:2 * sp + 2, :], ps[:])
            im["Q"], im["K"], im["VT"] = Q, K, VT

    def emit_scores(im):
        i = im["i"]
        Q, K = im["Q"], im["K"]
        with nc.named_scope(f"scores{i}"):
            PT = pPT.tile([P, NSB, NIC, FC], F8, name=f"PT{i}", tag="PT")
            for jb in range(NSB):
                ps = pmm.tile([P, NIC, FC], F32, name=f"sps{i}_{jb}", tag="mm")
                for kp in range(NKP):
                    lhs = K[:, 2 * kp:2 * kp + 2, jb // NJP,
                            (jb % NJP) * P:(jb % NJP + 1) * P]
                    for ic in range(NIC):
                        _mm8(nc, ps[:, ic], lhs,
                             Q[:, 2 * kp:2 * kp + 2, ic],
                             start=(kp == 0), stop=(kp == NKP - 1))
                nc.scalar.activation(PT[:, jb], ps[:], AF.Exp, bias=zb[:],
                                     scale=SM_SCALE / (SW * SW))
            im["PT"] = PT

    def emit_den(im):
        # den + reciprocal; the broadcast to all partitions is emitted later
        # (emit_attn_out) so the DVE chain hides under PE matmul work.
        i = im["i"]
        PT = im["PT"]
        with nc.named_scope(f"den{i}"):
            recip_dt = pS.tile([1, NIC, FC], DT, name=f"recipdt{i}", tag="recipdt")
            den = pden.tile([1, NIC, FC], F32, name=f"den{i}", tag="den")
            for ic in range(NIC):
                for jp in range(NJP):
                    _mm8(nc, den[:, ic], ones8[:],
                         PT[:, 2 * jp:2 * jp + 2, ic],
                         start=(jp == 0), stop=(jp == NJP - 1))
            recip = pS.tile([1, NIC, FC], F32, name=f"recip{i}", tag="recip")
            nc.vector.reciprocal_approx_fast(recip[:], den[:])
            # SR folds the extra NUM scale so NUM lands at 64*num
            nc.vector.tensor_scalar_mul(recip_dt[:], recip[:], SR)
            im["recip_dt"] = recip_dt

    def emit_attn_out(im):
        i = im["i"]
        X16, VT, PT = im["X16"], im["VT"], im["PT"]
        with nc.named_scope(f"attnout{i}"):
            NUM = pNUM.tile([P, NCT, NIC, FC], F8, name=f"NUM{i}", tag="NUM")
            recipb = pRB.tile([P, NIC, FC], F32, name=f"recipb{i}", tag="recipb")
            # recip broadcast first: recip_dt has long been ready (emit_den),
            # so recipb is available by the first NUM eviction
            rb = prb.tile([P, NIC, FC], F32, name=f"rb{i}", tag="aux")
            for ic in range(NIC):
                nc.tensor.matmul(rb[:, ic], ones_row[:], im["recip_dt"][:, ic],
                                 start=True, stop=True)
            nc.vector.tensor_copy(recipb[:], rb[:])
            for cb in range(NCT):
                ps = pmm.tile([P, NIC, FC], F32, name=f"nps{i}_{cb}", tag="mm")
                for jp in range(NJP):
                    lhs = VT[:, 2 * jp:2 * jp + 2, cb * P:(cb + 1) * P]
                    for ic in range(NIC):
                        _mm8(nc, ps[:, ic], lhs,
                             PT[:, 2 * jp:2 * jp + 2, ic],
                             start=(jp == 0), stop=(jp == NJP - 1))
                nc.vector.tensor_mul(NUM[:, cb], ps[:], recipb[:])
            # proj (+2048*(bo+wo@bv)) + 2048*residual from PSUM, then store
            OUTT = pOUT.tile([P, NCT, NIC, FC], DT, name=f"OUT{i}", tag="OUT")
            for ob in range(NCT):
                ps = pmm.tile([P, NIC, FC], F32, name=f"pps{i}_{ob}", tag="mm")
                for kp in range(NKP):
                    lhs = w_sb["wot"][:, 2 * kp:2 * kp + 2, ob * P:(ob + 1) * P]
                    for ic in range(NIC):
                        _mm8(nc, ps[:, ic], lhs,
                             NUM[:, 2 * kp:2 * kp + 2, ic],
                             start=(kp == 0), stop=(kp == NKP - 1))
                nc.vector.scalar_tensor_tensor(OUTT[:, ob], ps[:],
                                               bo_sb[:, ob:ob + 1], X16[:, ob],
                                               OP.add, OP.add)
                # stores on the HWDGE queues (SWDGE via gpsimd runs ~65GB/s)
                (nc.sync if ob % 2 == 0 else nc.scalar).dma_start(
                    io["out"][i, :, ob * HW:(ob + 1) * HW], OUTT[:, ob])

    ims = [new_img(i) for i in range(BPC)]
    a, b = ims
    # Emission order = per-engine program order. norm(a) before stats(b) so
    # the DVE GroupNorm chain for image a never queues behind image-b's DMA;
    # den(a) after qkv(b) so the exp(a) tail and the reciprocal chain hide
    # under qkv(b)'s matmuls; den(b) before attnout(a) so recip(b) lands on
    # DVE ahead of the NUM/OUT eviction backlog.
    emit_load16(a)
    emit_load16(b)
    emit_stats(a)
    emit_norm(a, fast=True)
    emit_stats(b)
    emit_norm(b, fast=False)
    emit_qkv(a)
    emit_scores(a)
    emit_qkv(b)
    emit_den(a)
    emit_scores(b)
    emit_den(b)
    emit_attn_out(a)
    emit_attn_out(b)


def _build():
    if "nc" in _CACHE:
        return _CACHE["nc"]
    nc = bacc.Bacc("TRN2", target_bir_lowering=False, debug=False, num_devices=NCORES)
    io = {}
    io["x16"] = nc.dram_tensor("x16", [BPC, P, NCT * HW], DT, kind="ExternalInput").ap()
    for wname in ("wqt", "wkt", "wvt", "wot"):
        io[wname] = nc.dram_tensor(wname, [P, NCT, C], F8, kind="ExternalInput").ap()
    io["cvec"] = nc.dram_tensor("cvec", [P, 5 * NCT + GPT], F32,
                                kind="ExternalInput").ap()
    io["gselT"] = nc.dram_tensor("gselT", [GPT, P], F32, kind="ExternalInput").ap()
    io["out"] = nc.dram_tensor("out", [BPC, P, NCT * HW], DT, kind="ExternalOutput").ap()

    with tile.TileContext(nc) as tc:
        with ExitStack() as ctx:
            _emit(ctx, tc, io)
    nc.compile()
    _CACHE["nc"] = nc
    return nc


def _col_layout(v, scale=1.0):
    # (C,) -> (P, NCT): column ct holds channels [ct*128, (ct+1)*128)
    return np.ascontiguousarray(
        (np.asarray(v, np.float32) * scale).reshape(NCT, P).T)


def _run(inputs, trace=False, **run_kwargs):
    x = np.asarray(inputs["x"], np.float32).reshape(B, C, HW)

    def _wpack(w):
        # w (c_out, c_in) -> [P, NCT, C] fp8 of 32*w.T
        wt = (np.asarray(w, np.float32).T * SW).astype(F8_NP)
        return np.ascontiguousarray(wt.reshape(NCT, P, C).transpose(1, 0, 2))

    wdt = {n: _wpack(inputs[s])
           for n, s in (("wqt", "wq"), ("wkt", "wk"), ("wvt", "wv"), ("wot", "wo"))}
    pidx = np.arange(P)
    gsel = (pidx[:, None] // CPG == np.arange(GPT)[None, :]).astype(np.float32)
    # bv never appears on-chip: sum_j P = den exactly, so it lands as wo @ bv
    bo_eff = (np.asarray(inputs["bo"], np.float32)
              + np.asarray(inputs["wo"], np.float32)
              @ np.asarray(inputs["bv"], np.float32))
    cvec = np.concatenate([_col_layout(inputs["bq"], SW),
                           _col_layout(inputs["bk"], SW),
                           _col_layout(bo_eff, SX),
                           _col_layout(inputs["gn_scale"]),
                           _col_layout(inputs["gn_bias"]), gsel], axis=1)
    common = {
        **wdt,
        "cvec": np.ascontiguousarray(cvec),
        "gselT": np.ascontiguousarray(gsel.T),
    }
    # 2048*x in bf16 (exact exponent shift), packed [B, P, NCT*HW]
    x16 = (x * SX).astype(DT_NP)
    x16p = x16.reshape(B, NCT, P, HW).transpose(0, 2, 1, 3).reshape(B, P, NCT * HW)
    in_maps = [{"x16": np.ascontiguousarray(x16p[m * BPC:(m + 1) * BPC]), **common}
               for m in range(NCORES)]
    nc = _build()
    res = run_bass_kernel_spmd(nc, in_maps, core_ids=list(range(NCORES)),
                               trace=trace, **run_kwargs)
    out = np.concatenate([r["out"] for r in res.results], axis=0)
    # [B, P, NCT*HW] -> [B, C, HW], undo the 2048x scale
    out = out.reshape(B, P, NCT, HW).transpose(0, 2, 1, 3).reshape(B, C, HW)
    out = out.astype(np.float32) * (1.0 / SX)
    return out.reshape(B, C, H, W), res


def kernel(**inputs):
    out, _ = _run(inputs)
    return out
